# revision 19
# baseline (speedup 1.0000x reference)
"""AngleLoss (HANDS17 bone-angle loss) on 8 TRN2 NeuronCores.

Math (per batch element b, bone pair (i0, i1)):
    v1 = pred[b, i0, :2] - pred[b, i1, :2]
    v2 = gt[b, i0, :2]   - gt[b, i1, :2]
    t  = |v1 . v2| / (|v1| |v2|)
    loss = mean over (b, pair) of (1 - t)

Strategy: pure data parallel over the batch. The loss only reads the
uv coordinates (42 of 63 floats) and the whole pipeline is bf16 on
device anyway (tolerance 2e-2; the previous f32 version cast to bf16
as its first op), so the host pre-packs each core's shard as uv-only
bf16 [BL, 42]. That cuts DMA traffic 3x (33 MB -> 11 MB per core) and
removes the on-device cast pass entirely; the kernel shifts from
DMA-bound (~96 us floor) to engine-bound (~70 us).

Engine assignment (measured rates, ns per max-AP element: DVE 2x
contiguous bf16 0.56, uv-pair subs 0.79, stride-2 pair add 1.11,
broadcast-operand sub 2.45 on DVE but ~3.0 on Pool; ACT ~0.9; Pool
~3.0). DVE & Pool share SBUF ports and ops co-touching the SAME tile
from different engines lose ~2x, so stages are staggered so no two
engines stream one tile in the same pipeline cycle:

  A(k):  DMA both tensors into one bf16 tile u[(t c), j, xy]
         (pred rows 0:C, gt C:2C); chain subs 1-3 (DVE, packed pairs)
  S0(k): root fan-out sub 0 (Pool, broadcast operand), queued behind
         den(k-3) so it runs while DVE works on small tiles
  P(k-1):  prod = v1*v2 (DVE 2x) one cycle later
  M(k-2):  sq = dc^2 (ACT), TRANSPOSED [xy-outer] out, one more cycle
           later -> never co-reads dc with prod or the subs
  B1(k-3): dot (DVE stride-2 add), nadd (DVE 2x), den = n1*n2 (Pool),
           a = |dot| (ACT, emitted after M so it never head-blocks)
  B2(k-4): e = exp(-0.5 ln(den+eps)) (ACT; Rsqrt banned in bass),
           t = a*e (DVE), ones-matmul accumulate (PE -> PSUM)

All pools triple-buffered; DMA runs ~3 tiles ahead. The host sums the
8 per-core partial sums and forms 1 - total/(B*P).
"""
import sys

sys.path.insert(0, "/opt/trn_rl_repo")

from contextlib import ExitStack

import ml_dtypes
import numpy as np

import concourse.bass as bass
import concourse.tile as tile
from concourse import mybir
from concourse.bass_utils import run_bass_kernel_spmd

B, J, DCOORD = 524288, 21, 3
NCORES = 8
P = 128                      # SBUF partitions
FU = J * 2                   # 42 uv floats per batch element
NPAIR = 20

f32 = mybir.dt.float32
bf16 = mybir.dt.bfloat16
AF = mybir.ActivationFunctionType
BF16 = ml_dtypes.bfloat16


def _split_excess_waits(nc, max_waits: int = 1) -> int:
    """The staged neuronxcc rejects instructions with more than one
    semaphore wait. Same-engine instructions run in order, so excess
    waits move onto preceding NoOps on the same engine."""
    n_split = 0
    for b in nc.m.functions[0].blocks:
        insts = b.instructions
        out = []
        changed = False
        for inst in insts:
            si = getattr(inst, "sync_info", None)
            waits = list(si.on_wait) if si is not None and si.on_wait else []
            if len(waits) > max_waits:
                extra, keep = waits[:-max_waits], waits[-max_waits:]
                while extra:
                    grp, extra = extra[:max_waits], extra[max_waits:]
                    nop = mybir.InstNoOp(
                        name=f"I-waitsplit-{n_split}", engine=inst.engine
                    )
                    nop.sync_info = mybir.SyncInfo(on_wait=grp, on_update=[])
                    out.append(nop)
                    n_split += 1
                inst.sync_info = mybir.SyncInfo(
                    on_wait=keep, on_update=list(si.on_update)
                )
                changed = True
            out.append(inst)
        if changed:
            insts[:] = out
    return n_split


def build_nc(tiles) -> bass.Bass:
    """One core's kernel. `tiles` is the list of per-tile batch counts C
    (batch elements per partition); total batch = P * sum(tiles)."""
    SC = sum(tiles)
    n_t = len(tiles)
    nc = bass.Bass()
    # partition-major layout: row p holds batch elements p*SC..(p+1)*SC-1,
    # so every per-tile DMA is a contiguous multi-KB span per partition
    # (84-byte row-sized runs would halve DMA bandwidth)
    x_ext = nc.declare_dram_parameter("jt_uvd_pred", [P, SC * FU], bf16, isOutput=False)
    g_ext = nc.declare_dram_parameter("jt_uvd_gt", [P, SC * FU], bf16, isOutput=False)
    out_ext = nc.declare_dram_parameter("out", [1, 1], f32, isOutput=True)
    NFMAX = NPAIR * max(tiles)

    with tile.TileContext(nc) as tc, ExitStack() as ctx:
        ins_pool = ctx.enter_context(tc.tile_pool(name="ins", bufs=6))
        mid_pool = ctx.enter_context(tc.tile_pool(name="mid", bufs=4))
        small_pool = ctx.enter_context(tc.tile_pool(name="small", bufs=4))
        const_pool = ctx.enter_context(tc.tile_pool(name="const", bufs=1))
        psum_pool = ctx.enter_context(tc.tile_pool(name="psum", bufs=1, space="PSUM"))

        ones = const_pool.tile([P, 1], bf16)
        nc.vector.memset(ones[:], 1.0)
        # bf16-rounded joints can collide -> exact-zero bones -> den=0;
        # ln(den+eps) keeps those pairs at t = 0*huge = 0 instead of NaN
        eps = const_pool.tile([P, 1], f32)
        nc.vector.memset(eps[:], 1e-30)

        # PSUM accumulators for the batch reduction, <=512 f32 per bank.
        psums = []
        off = 0
        while off < NFMAX:
            w = min(512, NFMAX - off)
            ps = psum_pool.tile([1, w], f32, name=f"ps{off}", tag=f"ps{off}")
            nc.vector.memset(ps[:], 0.0)
            psums.append((off, w, ps))
            off += w
        last_user = {}
        for i, C in enumerate(tiles):
            for k, (poff, w, ps) in enumerate(psums):
                if NPAIR * C > poff:
                    last_user[k] = i

        st = {}
        b0 = 0

        def emit_a(i):
            nonlocal b0
            C = tiles[i]
            FD = C * FU
            xv = x_ext[:, b0 : b0 + FD]
            gv = g_ext[:, b0 : b0 + FD]
            b0 += FD

            # bf16 uv landing tile: rows 0:C pred, C:2C gt
            u = ins_pool.tile([P, 2 * FD], bf16, tag="u")
            nc.sync.dma_start(out=u[:, 0:FD], in_=xv)
            nc.sync.dma_start(out=u[:, FD : 2 * FD], in_=gv)
            uv = u[:].rearrange("p (c j k) -> p c j k", j=J, k=2)

            # chain-bone gathers on DVE (all packed 4B uv pairs)
            dc = mid_pool.tile([P, 2 * C, NPAIR, 2], bf16, tag="dc")
            subs = [
                (5, uv[:, :, 1:6, :], uv[:, :, 6:19:3, :]),
                (10, uv[:, :, 6:19:3, :], uv[:, :, 7:20:3, :]),
                (15, uv[:, :, 7:20:3, :], uv[:, :, 8:21:3, :]),
            ]
            for s0, in0, in1 in subs:
                nc.vector.tensor_sub(out=dc[:, :, s0 : s0 + 5, :], in0=in0, in1=in1)
            st[i] = {"C": C, "uv": uv, "dc": dc}

        def emit_s0(i):
            # root fan-out (broadcast operand: 3x slower on DVE, normal
            # rate on Pool). Queued behind den(k-3) so it doesn't touch
            # u/dc while the DVE subs stream them.
            d = st[i]
            C, uv, dc = d["C"], d["uv"], d["dc"]
            root = uv[:, :, 0:1, :].broadcast_to([P, 2 * C, 5, 2])
            nc.gpsimd.tensor_sub(
                out=dc[:, :, 0:5, :], in0=root, in1=uv[:, :, 1:6, :]
            )

        def emit_prod(i):
            d = st[i]
            C, dc = d["C"], d["dc"]
            pr = mid_pool.tile([P, C, NPAIR, 2], bf16, tag="pr")
            nc.vector.tensor_mul(
                out=pr[:].rearrange("p c q k -> p (c q k)"),
                in0=dc[:, 0:C].rearrange("p c q k -> p (c q k)"),
                in1=dc[:, C : 2 * C].rearrange("p c q k -> p (c q k)"),
            )
            d["pr"] = pr

        def emit_m(i):
            d = st[i]
            C, dc = d["C"], d["dc"]
            s = mid_pool.tile([P, 2, 2 * C, NPAIR], bf16, tag="s")
            nc.scalar.activation(
                out=s[:], in_=dc[:].rearrange("p c q k -> p k c q"), func=AF.Square
            )
            d["s"] = s

        def emit_b1(i):
            d = st[i]
            C, pr, s = d["C"], d["pr"], d["s"]
            dot = small_pool.tile([P, C, NPAIR], bf16, tag="dot")
            nc.vector.tensor_add(out=dot[:], in0=pr[:, :, :, 0], in1=pr[:, :, :, 1])
            n = small_pool.tile([P, 2 * C, NPAIR], bf16, tag="n")
            nc.vector.tensor_add(
                out=n[:].rearrange("p c q -> p (c q)"),
                in0=s[:, 0].rearrange("p c q -> p (c q)"),
                in1=s[:, 1].rearrange("p c q -> p (c q)"),
            )
            den = small_pool.tile([P, C, NPAIR], bf16, tag="den")
            nc.gpsimd.tensor_mul(
                out=den[:].rearrange("p c q -> p (c q)"),
                in0=n[:, 0:C].rearrange("p c q -> p (c q)"),
                in1=n[:, C : 2 * C].rearrange("p c q -> p (c q)"),
            )
            d["dot"], d["den"] = dot, den

        def emit_abs(i):
            d = st[i]
            C, dot = d["C"], d["dot"]
            a = small_pool.tile([P, NPAIR * C], bf16, tag="a")
            nc.scalar.activation(
                out=a[:], in_=dot[:].rearrange("p c q -> p (c q)"), func=AF.Abs
            )
            d["a"] = a

        def emit_b2(i):
            d = st.pop(i)
            C, den, a = d["C"], d["den"], d["a"]
            NF = NPAIR * C
            lg = small_pool.tile([P, NF], bf16, tag="lg")
            nc.scalar.activation(
                out=lg[:],
                in_=den[:].rearrange("p c q -> p (c q)"),
                func=AF.Ln,
                bias=eps[:],
            )
            e = small_pool.tile([P, NF], bf16, tag="e")
            nc.scalar.activation(out=e[:], in_=lg[:], func=AF.Exp, scale=-0.5)
            t = small_pool.tile([P, NF], bf16, tag="t")
            nc.vector.tensor_mul(out=t[:], in0=a[:], in1=e[:])

            for k, (poff, w, ps) in enumerate(psums):
                if NF <= poff:
                    continue
                ww = min(w, NF - poff)
                nc.tensor.matmul(
                    out=ps[:, 0:ww],
                    lhsT=ones[:],
                    rhs=t[:, poff : poff + ww],
                    start=False,
                    stop=(last_user[k] == i),
                    skip_group_check=True,
                )

        # 5-deep software pipeline:
        # A(k)+S0(k) | P(k-1) | M(k-2) | B1(k-3)+ABS(k-3) | B2(k-4)
        for k in range(n_t + 4):
            if k < n_t:
                emit_a(k)
            if 4 <= k:
                emit_b2(k - 4)
            if 3 <= k <= n_t + 2:
                emit_b1(k - 3)
            if 1 <= k <= n_t:
                emit_prod(k - 1)
            if k < n_t:
                emit_s0(k)
            if 2 <= k <= n_t + 1:
                emit_m(k - 2)
            if 3 <= k <= n_t + 2:
                emit_abs(k - 3)

        # Tail: reduce each PSUM bank directly (DVE reads PSUM), then the
        # tiny per-bank sums, then DMA the scalar out
        t3 = const_pool.tile([1, len(psums)], f32)
        for k, (poff, w, ps) in enumerate(psums):
            nc.vector.tensor_reduce(
                out=t3[:, k : k + 1],
                in_=ps[:],
                op=mybir.AluOpType.add,
                axis=mybir.AxisListType.X,
            )
        total = const_pool.tile([1, 1], f32)
        nc.vector.tensor_reduce(
            out=total[:], in_=t3[:], op=mybir.AluOpType.add, axis=mybir.AxisListType.X
        )
        nc.sync.dma_start(out=out_ext[:], in_=total[:])

    return nc


_NC_CACHE: dict = {}

DEFAULT_TILES = (16, 32, 48, 48, 48, 48, 48, 48, 48, 48, 32, 32, 16)


def _get_nc(tiles) -> bass.Bass:
    key = tuple(tiles)
    if key not in _NC_CACHE:
        nc = build_nc(list(tiles))
        _split_excess_waits(nc)
        _NC_CACHE[key] = nc
    return _NC_CACHE[key]


def kernel(jt_uvd_pred, jt_uvd_gt, _tiles=DEFAULT_TILES, _trace: bool = False):
    pred = np.asarray(jt_uvd_pred)
    gt = np.asarray(jt_uvd_gt)
    Btot = pred.shape[0]
    assert pred.shape == (Btot, J, DCOORD) and gt.shape == (Btot, J, DCOORD)
    bl = P * sum(_tiles)
    assert bl * NCORES == Btot, (Btot, _tiles)

    # Host-side shard prep: uv coords only, rounded to bf16 (the device
    # pipeline is bf16 regardless; this also cuts DMA traffic 3x), in
    # partition-major [P, SC*42] layout for contiguous DMA spans.
    sc = sum(_tiles)
    pred_uv = np.ascontiguousarray(pred[:, :, :2]).astype(BF16).reshape(Btot, FU)
    gt_uv = np.ascontiguousarray(gt[:, :, :2]).astype(BF16).reshape(Btot, FU)

    nc = _get_nc(_tiles)
    in_maps = []
    for c in range(NCORES):
        sl = slice(c * bl, (c + 1) * bl)
        in_maps.append(
            {
                "jt_uvd_pred": pred_uv[sl].reshape(P, sc * FU),
                "jt_uvd_gt": gt_uv[sl].reshape(P, sc * FU),
            }
        )
    res = run_bass_kernel_spmd(
        nc, in_maps, core_ids=list(range(NCORES)), trace=_trace
    )
    total = sum(float(res.results[i]["out"][0, 0]) for i in range(NCORES))
    loss = 1.0 - total / (Btot * NPAIR)
    out = np.float32(loss)
    if _trace:
        return out, res
    return out


# revision 21
# speedup vs baseline: 1.2369x; 1.2369x over previous
"""AngleLoss (HANDS17 bone-angle loss) on 8 TRN2 NeuronCores.

Math (per batch element b, bone pair (i0, i1)):
    v1 = pred[b, i0, :2] - pred[b, i1, :2]
    v2 = gt[b, i0, :2]   - gt[b, i1, :2]
    t  = |v1 . v2| / (|v1| |v2|)
    loss = mean over (b, pair) of (1 - t)

Strategy: pure data parallel over the batch. The loss only reads the
uv coordinates (42 of 63 floats) and the whole pipeline is bf16 on
device anyway (tolerance 2e-2; the previous f32 version cast to bf16
as its first op), so the host pre-packs each core's shard as uv-only
bf16 [BL, 42]. That cuts DMA traffic 3x (33 MB -> 11 MB per core) and
removes the on-device cast pass entirely; the kernel shifts from
DMA-bound (~96 us floor) to engine-bound (~70 us).

Engine assignment (measured rates, ns per max-AP element: DVE 2x
contiguous bf16 0.56, uv-pair subs 0.79, stride-2 pair add 1.11,
broadcast-operand sub 2.45 on DVE but ~3.0 on Pool; ACT ~0.9; Pool
~3.0). DVE & Pool share SBUF ports and ops co-touching the SAME tile
from different engines lose ~2x, so stages are staggered so no two
engines stream one tile in the same pipeline cycle:

  A(k):  DMA both tensors into one bf16 tile u[(t c), j, xy]
         (pred rows 0:C, gt C:2C); chain subs 1-3 (DVE, packed pairs)
  S0(k): root fan-out sub 0 (Pool, broadcast operand), queued behind
         den(k-3) so it runs while DVE works on small tiles
  P(k-1):  prod = v1*v2 (DVE 2x) one cycle later
  M(k-2):  sq = dc^2 (ACT), TRANSPOSED [xy-outer] out, one more cycle
           later -> never co-reads dc with prod or the subs
  B1(k-3): dot (DVE stride-2 add), nadd (DVE 2x), den = n1*n2 (Pool),
           a = |dot| (ACT, emitted after M so it never head-blocks)
  B2(k-4): e = exp(-0.5 ln(den+eps)) (ACT; Rsqrt banned in bass),
           t = a*e (DVE), ones-matmul accumulate (PE -> PSUM)

All pools triple-buffered; DMA runs ~3 tiles ahead. The host sums the
8 per-core partial sums and forms 1 - total/(B*P).
"""
import sys

sys.path.insert(0, "/opt/trn_rl_repo")

from contextlib import ExitStack

import ml_dtypes
import numpy as np

import concourse.bass as bass
import concourse.tile as tile
from concourse import mybir
from concourse.bass_utils import run_bass_kernel_spmd

B, J, DCOORD = 524288, 21, 3
NCORES = 8
P = 128                      # SBUF partitions
FU = J * 2                   # 42 uv floats per batch element
NPAIR = 20

f32 = mybir.dt.float32
bf16 = mybir.dt.bfloat16
AF = mybir.ActivationFunctionType
BF16 = ml_dtypes.bfloat16


def _split_excess_waits(nc, max_waits: int = 1) -> int:
    """The staged neuronxcc rejects instructions with more than one
    semaphore wait. Same-engine instructions run in order, so excess
    waits move onto preceding NoOps on the same engine."""
    n_split = 0
    for b in nc.m.functions[0].blocks:
        insts = b.instructions
        out = []
        changed = False
        for inst in insts:
            si = getattr(inst, "sync_info", None)
            waits = list(si.on_wait) if si is not None and si.on_wait else []
            if len(waits) > max_waits:
                extra, keep = waits[:-max_waits], waits[-max_waits:]
                while extra:
                    grp, extra = extra[:max_waits], extra[max_waits:]
                    nop = mybir.InstNoOp(
                        name=f"I-waitsplit-{n_split}", engine=inst.engine
                    )
                    nop.sync_info = mybir.SyncInfo(on_wait=grp, on_update=[])
                    out.append(nop)
                    n_split += 1
                inst.sync_info = mybir.SyncInfo(
                    on_wait=keep, on_update=list(si.on_update)
                )
                changed = True
            out.append(inst)
        if changed:
            insts[:] = out
    return n_split


def build_nc(tiles) -> bass.Bass:
    """One core's kernel. `tiles` is the list of per-tile batch counts C
    (batch elements per partition); total batch = P * sum(tiles)."""
    SC = sum(tiles)
    n_t = len(tiles)
    nc = bass.Bass()
    # partition-major layout: row p holds batch elements p*SC..(p+1)*SC-1,
    # so every per-tile DMA is a contiguous multi-KB span per partition
    # (84-byte row-sized runs would halve DMA bandwidth)
    x_ext = nc.declare_dram_parameter("jt_uvd_pred", [P, SC * FU], bf16, isOutput=False)
    g_ext = nc.declare_dram_parameter("jt_uvd_gt", [P, SC * FU], bf16, isOutput=False)
    out_ext = nc.declare_dram_parameter("out", [1, 1], f32, isOutput=True)
    NFMAX = NPAIR * max(tiles)

    with tile.TileContext(nc) as tc, ExitStack() as ctx:
        ins_pool = ctx.enter_context(tc.tile_pool(name="ins", bufs=6))
        mid_pool = ctx.enter_context(tc.tile_pool(name="mid", bufs=4))
        small_pool = ctx.enter_context(tc.tile_pool(name="small", bufs=4))
        const_pool = ctx.enter_context(tc.tile_pool(name="const", bufs=1))
        psum_pool = ctx.enter_context(tc.tile_pool(name="psum", bufs=1, space="PSUM"))

        ones = const_pool.tile([P, 1], bf16)
        nc.vector.memset(ones[:], 1.0)
        # bf16-rounded joints can collide -> exact-zero bones -> den=0;
        # ln(den+eps) keeps those pairs at t = 0*huge = 0 instead of NaN
        eps = const_pool.tile([P, 1], f32)
        nc.vector.memset(eps[:], 1e-30)

        # PSUM accumulators for the batch reduction, <=512 f32 per bank.
        psums = []
        off = 0
        while off < NFMAX:
            w = min(512, NFMAX - off)
            ps = psum_pool.tile([1, w], f32, name=f"ps{off}", tag=f"ps{off}")
            nc.vector.memset(ps[:], 0.0)
            psums.append((off, w, ps))
            off += w
        last_user = {}
        for i, C in enumerate(tiles):
            for k, (poff, w, ps) in enumerate(psums):
                if NPAIR * C > poff:
                    last_user[k] = i

        st = {}
        b0 = 0

        def emit_a(i):
            nonlocal b0
            C = tiles[i]
            FD = C * FU
            xv = x_ext[:, b0 : b0 + FD]
            gv = g_ext[:, b0 : b0 + FD]
            b0 += FD

            # bf16 uv landing tile: rows 0:C pred, C:2C gt
            u = ins_pool.tile([P, 2 * FD], bf16, tag="u")
            nc.sync.dma_start(out=u[:, 0:FD], in_=xv)
            nc.sync.dma_start(out=u[:, FD : 2 * FD], in_=gv)
            uv = u[:].rearrange("p (c j k) -> p c j k", j=J, k=2)

            # chain-bone gathers on DVE (all packed 4B uv pairs)
            dc = mid_pool.tile([P, 2 * C, NPAIR, 2], bf16, tag="dc")
            subs = [
                (5, uv[:, :, 1:6, :], uv[:, :, 6:19:3, :]),
                (10, uv[:, :, 6:19:3, :], uv[:, :, 7:20:3, :]),
                (15, uv[:, :, 7:20:3, :], uv[:, :, 8:21:3, :]),
            ]
            for s0, in0, in1 in subs:
                nc.vector.tensor_sub(out=dc[:, :, s0 : s0 + 5, :], in0=in0, in1=in1)
            st[i] = {"C": C, "uv": uv, "dc": dc}

        def emit_s0(i):
            # root fan-out (broadcast operand: 3x slower on DVE, normal
            # rate on Pool). Queued behind den(k-3) so it doesn't touch
            # u/dc while the DVE subs stream them.
            d = st[i]
            C, uv, dc = d["C"], d["uv"], d["dc"]
            root = uv[:, :, 0:1, :].broadcast_to([P, 2 * C, 5, 2])
            nc.gpsimd.tensor_sub(
                out=dc[:, :, 0:5, :], in0=root, in1=uv[:, :, 1:6, :]
            )

        def emit_prod(i):
            d = st[i]
            C, dc = d["C"], d["dc"]
            pr = mid_pool.tile([P, C, NPAIR, 2], bf16, tag="pr")
            nc.vector.tensor_mul(
                out=pr[:].rearrange("p c q k -> p (c q k)"),
                in0=dc[:, 0:C].rearrange("p c q k -> p (c q k)"),
                in1=dc[:, C : 2 * C].rearrange("p c q k -> p (c q k)"),
            )
            d["pr"] = pr

        def emit_m(i):
            d = st[i]
            C, dc = d["C"], d["dc"]
            s = mid_pool.tile([P, 2, 2 * C, NPAIR], bf16, tag="s")
            nc.scalar.activation(
                out=s[:], in_=dc[:].rearrange("p c q k -> p k c q"), func=AF.Square
            )
            d["s"] = s

        def emit_b1(i):
            # nadd FIRST in the DVE cycle (its input s is a cycle old)
            # so Pool's den fires early instead of convoying behind the
            # whole DVE queue
            d = st[i]
            C, pr, s = d["C"], d["pr"], d["s"]
            n = small_pool.tile([P, 2 * C, NPAIR], bf16, tag="n")
            nc.vector.tensor_add(
                out=n[:].rearrange("p c q -> p (c q)"),
                in0=s[:, 0].rearrange("p c q -> p (c q)"),
                in1=s[:, 1].rearrange("p c q -> p (c q)"),
            )
            dot = small_pool.tile([P, C, NPAIR], bf16, tag="dot")
            nc.vector.tensor_add(out=dot[:], in0=pr[:, :, :, 0], in1=pr[:, :, :, 1])
            den = small_pool.tile([P, C, NPAIR], bf16, tag="den")
            nc.gpsimd.tensor_mul(
                out=den[:].rearrange("p c q -> p (c q)"),
                in0=n[:, 0:C].rearrange("p c q -> p (c q)"),
                in1=n[:, C : 2 * C].rearrange("p c q -> p (c q)"),
            )
            d["dot"], d["den"] = dot, den

        def emit_abs(i):
            d = st[i]
            C, dot = d["C"], d["dot"]
            a = small_pool.tile([P, NPAIR * C], bf16, tag="a")
            nc.scalar.activation(
                out=a[:], in_=dot[:].rearrange("p c q -> p (c q)"), func=AF.Abs
            )
            d["a"] = a

        def emit_b2(i):
            d = st.pop(i)
            C, den, a = d["C"], d["den"], d["a"]
            NF = NPAIR * C
            lg = small_pool.tile([P, NF], bf16, tag="lg")
            nc.scalar.activation(
                out=lg[:],
                in_=den[:].rearrange("p c q -> p (c q)"),
                func=AF.Ln,
                bias=eps[:],
            )
            e = small_pool.tile([P, NF], bf16, tag="e")
            nc.scalar.activation(out=e[:], in_=lg[:], func=AF.Exp, scale=-0.5)
            t = small_pool.tile([P, NF], bf16, tag="t")
            nc.vector.tensor_mul(out=t[:], in0=a[:], in1=e[:])

            for k, (poff, w, ps) in enumerate(psums):
                if NF <= poff:
                    continue
                ww = min(w, NF - poff)
                nc.tensor.matmul(
                    out=ps[:, 0:ww],
                    lhsT=ones[:],
                    rhs=t[:, poff : poff + ww],
                    start=False,
                    stop=(last_user[k] == i),
                    skip_group_check=True,
                )

        # 5-deep software pipeline:
        # B1(k-3) | A(k)+S0(k) | B2(k-4) | P(k-1) | M(k-2) | ABS(k-3)
        # DVE queue: nadd, dot, subs123, t, prod; Pool: den, sub0;
        # ACT: ln, exp, sq, abs -- every queue head's input is >= one
        # cycle old, so no engine convoys behind another.
        for k in range(n_t + 4):
            if 3 <= k <= n_t + 2:
                emit_b1(k - 3)
            if k < n_t:
                emit_a(k)
            if 4 <= k:
                emit_b2(k - 4)
            if 1 <= k <= n_t:
                emit_prod(k - 1)
            if k < n_t:
                emit_s0(k)
            if 2 <= k <= n_t + 1:
                emit_m(k - 2)
            if 3 <= k <= n_t + 2:
                emit_abs(k - 3)

        # Tail: reduce each PSUM bank directly (DVE reads PSUM), then the
        # tiny per-bank sums, then DMA the scalar out
        t3 = const_pool.tile([1, len(psums)], f32)
        for k, (poff, w, ps) in enumerate(psums):
            nc.vector.tensor_reduce(
                out=t3[:, k : k + 1],
                in_=ps[:],
                op=mybir.AluOpType.add,
                axis=mybir.AxisListType.X,
            )
        total = const_pool.tile([1, 1], f32)
        nc.vector.tensor_reduce(
            out=total[:], in_=t3[:], op=mybir.AluOpType.add, axis=mybir.AxisListType.X
        )
        nc.sync.dma_start(out=out_ext[:], in_=total[:])

    return nc


_NC_CACHE: dict = {}

DEFAULT_TILES = (16, 32, 48, 48, 48, 48, 48, 48, 48, 48, 32, 32, 16)


def _get_nc(tiles) -> bass.Bass:
    key = tuple(tiles)
    if key not in _NC_CACHE:
        nc = build_nc(list(tiles))
        _split_excess_waits(nc)
        _NC_CACHE[key] = nc
    return _NC_CACHE[key]


def kernel(jt_uvd_pred, jt_uvd_gt, _tiles=DEFAULT_TILES, _trace: bool = False):
    pred = np.asarray(jt_uvd_pred)
    gt = np.asarray(jt_uvd_gt)
    Btot = pred.shape[0]
    assert pred.shape == (Btot, J, DCOORD) and gt.shape == (Btot, J, DCOORD)
    bl = P * sum(_tiles)
    assert bl * NCORES == Btot, (Btot, _tiles)

    # Host-side shard prep: uv coords only, rounded to bf16 (the device
    # pipeline is bf16 regardless; this also cuts DMA traffic 3x), in
    # partition-major [P, SC*42] layout for contiguous DMA spans.
    sc = sum(_tiles)
    pred_uv = np.ascontiguousarray(pred[:, :, :2]).astype(BF16).reshape(Btot, FU)
    gt_uv = np.ascontiguousarray(gt[:, :, :2]).astype(BF16).reshape(Btot, FU)

    nc = _get_nc(_tiles)
    in_maps = []
    for c in range(NCORES):
        sl = slice(c * bl, (c + 1) * bl)
        in_maps.append(
            {
                "jt_uvd_pred": pred_uv[sl].reshape(P, sc * FU),
                "jt_uvd_gt": gt_uv[sl].reshape(P, sc * FU),
            }
        )
    res = run_bass_kernel_spmd(
        nc, in_maps, core_ids=list(range(NCORES)), trace=_trace
    )
    total = sum(float(res.results[i]["out"][0, 0]) for i in range(NCORES))
    loss = 1.0 - total / (Btot * NPAIR)
    out = np.float32(loss)
    if _trace:
        return out, res
    return out


# revision 22
# speedup vs baseline: 1.2939x; 1.0460x over previous
"""AngleLoss (HANDS17 bone-angle loss) on 8 TRN2 NeuronCores.

Math (per batch element b, bone pair (i0, i1)):
    v1 = pred[b, i0, :2] - pred[b, i1, :2]
    v2 = gt[b, i0, :2]   - gt[b, i1, :2]
    t  = |v1 . v2| / (|v1| |v2|)
    loss = mean over (b, pair) of (1 - t)

Strategy: pure data parallel over the batch. The loss only reads the
uv coordinates (42 of 63 floats) and the whole pipeline is bf16 on
device anyway (tolerance 2e-2; the previous f32 version cast to bf16
as its first op), so the host pre-packs each core's shard as uv-only
bf16 [BL, 42]. That cuts DMA traffic 3x (33 MB -> 11 MB per core) and
removes the on-device cast pass entirely; the kernel shifts from
DMA-bound (~96 us floor) to engine-bound (~70 us).

Engine assignment (measured rates, ns per max-AP element: DVE 2x
contiguous bf16 0.56, uv-pair subs 0.79, stride-2 pair add 1.11,
broadcast-operand sub 2.45 on DVE but ~3.0 on Pool; ACT ~0.9; Pool
~3.0). DVE & Pool share SBUF ports and ops co-touching the SAME tile
from different engines lose ~2x, so stages are staggered so no two
engines stream one tile in the same pipeline cycle:

  A(k):  DMA both tensors into one bf16 tile u[(t c), j, xy]
         (pred rows 0:C, gt C:2C); chain subs 1-3 (DVE, packed pairs)
  S0(k): root fan-out sub 0 (Pool, broadcast operand), queued behind
         den(k-3) so it runs while DVE works on small tiles
  P(k-1):  prod = v1*v2 (DVE 2x) one cycle later
  M(k-2):  sq = dc^2 (ACT), TRANSPOSED [xy-outer] out, one more cycle
           later -> never co-reads dc with prod or the subs
  B1(k-3): dot (DVE stride-2 add), nadd (DVE 2x), den = n1*n2 (Pool),
           a = |dot| (ACT, emitted after M so it never head-blocks)
  B2(k-4): e = exp(-0.5 ln(den+eps)) (ACT; Rsqrt banned in bass),
           t = a*e (DVE), ones-matmul accumulate (PE -> PSUM)

All pools triple-buffered; DMA runs ~3 tiles ahead. The host sums the
8 per-core partial sums and forms 1 - total/(B*P).
"""
import sys

sys.path.insert(0, "/opt/trn_rl_repo")

from contextlib import ExitStack

import ml_dtypes
import numpy as np

import concourse.bass as bass
import concourse.tile as tile
from concourse import mybir
from concourse.bass_utils import run_bass_kernel_spmd

B, J, DCOORD = 524288, 21, 3
NCORES = 8
P = 128                      # SBUF partitions
J2 = 26                      # 21 joints + 5 ghost copies of joint 0
FU = J2 * 2                  # 52 uv bf16 values per batch element
NPAIR = 20

f32 = mybir.dt.float32
bf16 = mybir.dt.bfloat16
AF = mybir.ActivationFunctionType
BF16 = ml_dtypes.bfloat16


def _split_excess_waits(nc, max_waits: int = 1) -> int:
    """The staged neuronxcc rejects instructions with more than one
    semaphore wait. Same-engine instructions run in order, so excess
    waits move onto preceding NoOps on the same engine."""
    n_split = 0
    for b in nc.m.functions[0].blocks:
        insts = b.instructions
        out = []
        changed = False
        for inst in insts:
            si = getattr(inst, "sync_info", None)
            waits = list(si.on_wait) if si is not None and si.on_wait else []
            if len(waits) > max_waits:
                extra, keep = waits[:-max_waits], waits[-max_waits:]
                while extra:
                    grp, extra = extra[:max_waits], extra[max_waits:]
                    nop = mybir.InstNoOp(
                        name=f"I-waitsplit-{n_split}", engine=inst.engine
                    )
                    nop.sync_info = mybir.SyncInfo(on_wait=grp, on_update=[])
                    out.append(nop)
                    n_split += 1
                inst.sync_info = mybir.SyncInfo(
                    on_wait=keep, on_update=list(si.on_update)
                )
                changed = True
            out.append(inst)
        if changed:
            insts[:] = out
    return n_split


def build_nc(tiles) -> bass.Bass:
    """One core's kernel. `tiles` is the list of per-tile batch counts C
    (batch elements per partition); total batch = P * sum(tiles)."""
    SC = sum(tiles)
    n_t = len(tiles)
    nc = bass.Bass()
    # partition-major layout: row p holds batch elements p*SC..(p+1)*SC-1,
    # so every per-tile DMA is a contiguous multi-KB span per partition
    # (84-byte row-sized runs would halve DMA bandwidth)
    x_ext = nc.declare_dram_parameter("jt_uvd_pred", [P, SC * FU], bf16, isOutput=False)
    g_ext = nc.declare_dram_parameter("jt_uvd_gt", [P, SC * FU], bf16, isOutput=False)
    out_ext = nc.declare_dram_parameter("out", [1, 1], f32, isOutput=True)
    NFMAX = NPAIR * max(tiles)

    with tile.TileContext(nc) as tc, ExitStack() as ctx:
        ins_pool = ctx.enter_context(tc.tile_pool(name="ins", bufs=6))
        mid_pool = ctx.enter_context(tc.tile_pool(name="mid", bufs=4))
        small_pool = ctx.enter_context(tc.tile_pool(name="small", bufs=4))
        const_pool = ctx.enter_context(tc.tile_pool(name="const", bufs=1))
        psum_pool = ctx.enter_context(tc.tile_pool(name="psum", bufs=1, space="PSUM"))

        ones = const_pool.tile([P, 1], bf16)
        nc.vector.memset(ones[:], 1.0)
        # bf16-rounded joints can collide -> exact-zero bones -> den=0;
        # ln(den+eps) keeps those pairs at t = 0*huge = 0 instead of NaN
        eps = const_pool.tile([P, 1], f32)
        nc.vector.memset(eps[:], 1e-30)

        # PSUM accumulators for the batch reduction, <=512 f32 per bank.
        psums = []
        off = 0
        while off < NFMAX:
            w = min(512, NFMAX - off)
            ps = psum_pool.tile([1, w], f32, name=f"ps{off}", tag=f"ps{off}")
            nc.vector.memset(ps[:], 0.0)
            psums.append((off, w, ps))
            off += w
        last_user = {}
        for i, C in enumerate(tiles):
            for k, (poff, w, ps) in enumerate(psums):
                if NPAIR * C > poff:
                    last_user[k] = i

        st = {}
        b0 = 0

        def emit_a(i):
            nonlocal b0
            C = tiles[i]
            FD = C * FU
            xv = x_ext[:, b0 : b0 + FD]
            gv = g_ext[:, b0 : b0 + FD]
            b0 += FD

            # bf16 uv landing tile: rows 0:C pred, C:2C gt
            u = ins_pool.tile([P, 2 * FD], bf16, tag="u")
            nc.sync.dma_start(out=u[:, 0:FD], in_=xv)
            nc.sync.dma_start(out=u[:, FD : 2 * FD], in_=gv)
            uv = u[:].rearrange("p (c j k) -> p c j k", j=J2, k=2)

            # bone gathers on DVE (all packed 4B uv pairs). The root
            # fan-out uses the host-packed ghost copies of joint 0
            # (columns 21:26), so no broadcast operand is needed --
            # a stride-0 operand runs 3x slower on DVE.
            dc = mid_pool.tile([P, 2 * C, NPAIR, 2], bf16, tag="dc")
            subs = [
                (0, uv[:, :, 21:26, :], uv[:, :, 1:6, :]),
                (5, uv[:, :, 1:6, :], uv[:, :, 6:19:3, :]),
                (10, uv[:, :, 6:19:3, :], uv[:, :, 7:20:3, :]),
                (15, uv[:, :, 7:20:3, :], uv[:, :, 8:21:3, :]),
            ]
            for s0, in0, in1 in subs:
                nc.vector.tensor_sub(out=dc[:, :, s0 : s0 + 5, :], in0=in0, in1=in1)
            st[i] = {"C": C, "uv": uv, "dc": dc}

        def emit_prod(i):
            d = st[i]
            C, dc = d["C"], d["dc"]
            pr = mid_pool.tile([P, C, NPAIR, 2], bf16, tag="pr")
            nc.vector.tensor_mul(
                out=pr[:].rearrange("p c q k -> p (c q k)"),
                in0=dc[:, 0:C].rearrange("p c q k -> p (c q k)"),
                in1=dc[:, C : 2 * C].rearrange("p c q k -> p (c q k)"),
            )
            d["pr"] = pr

        def emit_m(i):
            d = st[i]
            C, dc = d["C"], d["dc"]
            s = mid_pool.tile([P, 2, 2 * C, NPAIR], bf16, tag="s")
            nc.scalar.activation(
                out=s[:], in_=dc[:].rearrange("p c q k -> p k c q"), func=AF.Square
            )
            d["s"] = s

        def emit_b1(i):
            # nadd FIRST in the DVE cycle (its input s is a cycle old)
            # so Pool's den fires early instead of convoying behind the
            # whole DVE queue
            d = st[i]
            C, pr, s = d["C"], d["pr"], d["s"]
            n = small_pool.tile([P, 2 * C, NPAIR], bf16, tag="n")
            nc.vector.tensor_add(
                out=n[:].rearrange("p c q -> p (c q)"),
                in0=s[:, 0].rearrange("p c q -> p (c q)"),
                in1=s[:, 1].rearrange("p c q -> p (c q)"),
            )
            dot = small_pool.tile([P, C, NPAIR], bf16, tag="dot")
            nc.vector.tensor_add(out=dot[:], in0=pr[:, :, :, 0], in1=pr[:, :, :, 1])
            den = small_pool.tile([P, C, NPAIR], bf16, tag="den")
            nc.gpsimd.tensor_mul(
                out=den[:].rearrange("p c q -> p (c q)"),
                in0=n[:, 0:C].rearrange("p c q -> p (c q)"),
                in1=n[:, C : 2 * C].rearrange("p c q -> p (c q)"),
            )
            d["dot"], d["den"] = dot, den

        def emit_abs(i):
            d = st[i]
            C, dot = d["C"], d["dot"]
            a = small_pool.tile([P, NPAIR * C], bf16, tag="a")
            nc.scalar.activation(
                out=a[:], in_=dot[:].rearrange("p c q -> p (c q)"), func=AF.Abs
            )
            d["a"] = a

        def emit_b2(i):
            d = st.pop(i)
            C, den, a = d["C"], d["den"], d["a"]
            NF = NPAIR * C
            lg = small_pool.tile([P, NF], bf16, tag="lg")
            nc.scalar.activation(
                out=lg[:],
                in_=den[:].rearrange("p c q -> p (c q)"),
                func=AF.Ln,
                bias=eps[:],
            )
            e = small_pool.tile([P, NF], bf16, tag="e")
            nc.scalar.activation(out=e[:], in_=lg[:], func=AF.Exp, scale=-0.5)
            t = small_pool.tile([P, NF], bf16, tag="t")
            nc.vector.tensor_mul(out=t[:], in0=a[:], in1=e[:])

            for k, (poff, w, ps) in enumerate(psums):
                if NF <= poff:
                    continue
                ww = min(w, NF - poff)
                nc.tensor.matmul(
                    out=ps[:, 0:ww],
                    lhsT=ones[:],
                    rhs=t[:, poff : poff + ww],
                    start=False,
                    stop=(last_user[k] == i),
                    skip_group_check=True,
                )

        # 5-deep software pipeline:
        # B1(k-3) | A(k)+S0(k) | B2(k-4) | P(k-1) | M(k-2) | ABS(k-3)
        # DVE queue: nadd, dot, subs123, t, prod; Pool: den, sub0;
        # ACT: ln, exp, sq, abs -- every queue head's input is >= one
        # cycle old, so no engine convoys behind another.
        for k in range(n_t + 4):
            if 3 <= k <= n_t + 2:
                emit_b1(k - 3)
            if k < n_t:
                emit_a(k)
            if 4 <= k:
                emit_b2(k - 4)
            if 1 <= k <= n_t:
                emit_prod(k - 1)
            if 2 <= k <= n_t + 1:
                emit_m(k - 2)
            if 3 <= k <= n_t + 2:
                emit_abs(k - 3)

        # Tail: reduce each PSUM bank directly (DVE reads PSUM), then the
        # tiny per-bank sums, then DMA the scalar out
        t3 = const_pool.tile([1, len(psums)], f32)
        for k, (poff, w, ps) in enumerate(psums):
            nc.vector.tensor_reduce(
                out=t3[:, k : k + 1],
                in_=ps[:],
                op=mybir.AluOpType.add,
                axis=mybir.AxisListType.X,
            )
        total = const_pool.tile([1, 1], f32)
        nc.vector.tensor_reduce(
            out=total[:], in_=t3[:], op=mybir.AluOpType.add, axis=mybir.AxisListType.X
        )
        nc.sync.dma_start(out=out_ext[:], in_=total[:])

    return nc


_NC_CACHE: dict = {}

DEFAULT_TILES = (16, 32, 48, 48, 48, 48, 48, 48, 48, 48, 32, 32, 16)


def _get_nc(tiles) -> bass.Bass:
    key = tuple(tiles)
    if key not in _NC_CACHE:
        nc = build_nc(list(tiles))
        _split_excess_waits(nc)
        _NC_CACHE[key] = nc
    return _NC_CACHE[key]


def kernel(jt_uvd_pred, jt_uvd_gt, _tiles=DEFAULT_TILES, _trace: bool = False):
    pred = np.asarray(jt_uvd_pred)
    gt = np.asarray(jt_uvd_gt)
    Btot = pred.shape[0]
    assert pred.shape == (Btot, J, DCOORD) and gt.shape == (Btot, J, DCOORD)
    bl = P * sum(_tiles)
    assert bl * NCORES == Btot, (Btot, _tiles)

    # Host-side shard prep: uv coords only, rounded to bf16 (the device
    # pipeline is bf16 regardless; this also cuts DMA traffic ~3x), with
    # joint 0 replicated 5x per row so the root fan-out subtract needs
    # no broadcast operand, in partition-major [P, SC*FU] layout for
    # contiguous DMA spans.
    sc = sum(_tiles)

    def pack(arr):
        a = np.ascontiguousarray(arr[:, :, :2]).astype(BF16)
        ghost = np.broadcast_to(a[:, 0:1, :], (Btot, 5, 2))
        return np.concatenate([a, ghost], axis=1).reshape(Btot, FU)

    pred_uv = pack(pred)
    gt_uv = pack(gt)

    nc = _get_nc(_tiles)
    in_maps = []
    for c in range(NCORES):
        sl = slice(c * bl, (c + 1) * bl)
        in_maps.append(
            {
                "jt_uvd_pred": pred_uv[sl].reshape(P, sc * FU),
                "jt_uvd_gt": gt_uv[sl].reshape(P, sc * FU),
            }
        )
    res = run_bass_kernel_spmd(
        nc, in_maps, core_ids=list(range(NCORES)), trace=_trace
    )
    total = sum(float(res.results[i]["out"][0, 0]) for i in range(NCORES))
    loss = 1.0 - total / (Btot * NPAIR)
    out = np.float32(loss)
    if _trace:
        return out, res
    return out


# revision 23
# speedup vs baseline: 1.3156x; 1.0168x over previous
"""AngleLoss (HANDS17 bone-angle loss) on 8 TRN2 NeuronCores.

Math (per batch element b, bone pair (i0, i1)):
    v1 = pred[b, i0, :2] - pred[b, i1, :2]
    v2 = gt[b, i0, :2]   - gt[b, i1, :2]
    t  = |v1 . v2| / (|v1| |v2|)
    loss = mean over (b, pair) of (1 - t)

Strategy: pure data parallel over the batch. The loss only reads the
uv coordinates (42 of 63 floats) and the whole pipeline is bf16 on
device anyway (tolerance 2e-2; the previous f32 version cast to bf16
as its first op), so the host pre-packs each core's shard as uv-only
bf16 [BL, 42]. That cuts DMA traffic 3x (33 MB -> 11 MB per core) and
removes the on-device cast pass entirely; the kernel shifts from
DMA-bound (~96 us floor) to engine-bound (~70 us).

Engine assignment (measured rates, ns per max-AP element: DVE 2x
contiguous bf16 0.56, uv-pair subs 0.79, stride-2 pair add 1.11,
broadcast-operand sub 2.45 on DVE but ~3.0 on Pool; ACT ~0.9; Pool
~3.0). DVE & Pool share SBUF ports and ops co-touching the SAME tile
from different engines lose ~2x, so stages are staggered so no two
engines stream one tile in the same pipeline cycle:

  A(k):  DMA both tensors into one bf16 tile u[(t c), j, xy]
         (pred rows 0:C, gt C:2C); chain subs 1-3 (DVE, packed pairs)
  S0(k): root fan-out sub 0 (Pool, broadcast operand), queued behind
         den(k-3) so it runs while DVE works on small tiles
  P(k-1):  prod = v1*v2 (DVE 2x) one cycle later
  M(k-2):  sq = dc^2 (ACT), TRANSPOSED [xy-outer] out, one more cycle
           later -> never co-reads dc with prod or the subs
  B1(k-3): dot (DVE stride-2 add), nadd (DVE 2x), den = n1*n2 (Pool),
           a = |dot| (ACT, emitted after M so it never head-blocks)
  B2(k-4): e = exp(-0.5 ln(den+eps)) (ACT; Rsqrt banned in bass),
           t = a*e (DVE), ones-matmul accumulate (PE -> PSUM)

All pools triple-buffered; DMA runs ~3 tiles ahead. The host sums the
8 per-core partial sums and forms 1 - total/(B*P).
"""
import sys

sys.path.insert(0, "/opt/trn_rl_repo")

from contextlib import ExitStack

import ml_dtypes
import numpy as np

import concourse.bass as bass
import concourse.tile as tile
from concourse import mybir
from concourse.bass_utils import run_bass_kernel_spmd

B, J, DCOORD = 524288, 21, 3
NCORES = 8
P = 128                      # SBUF partitions
J2 = 26                      # 21 joints + 5 ghost copies of joint 0
FU = J2 * 2                  # 52 uv bf16 values per batch element
NPAIR = 20

f32 = mybir.dt.float32
bf16 = mybir.dt.bfloat16
AF = mybir.ActivationFunctionType
BF16 = ml_dtypes.bfloat16


def _split_excess_waits(nc, max_waits: int = 1) -> int:
    """The staged neuronxcc rejects instructions with more than one
    semaphore wait. Same-engine instructions run in order, so excess
    waits move onto preceding NoOps on the same engine."""
    n_split = 0
    for b in nc.m.functions[0].blocks:
        insts = b.instructions
        out = []
        changed = False
        for inst in insts:
            si = getattr(inst, "sync_info", None)
            waits = list(si.on_wait) if si is not None and si.on_wait else []
            if len(waits) > max_waits:
                extra, keep = waits[:-max_waits], waits[-max_waits:]
                while extra:
                    grp, extra = extra[:max_waits], extra[max_waits:]
                    nop = mybir.InstNoOp(
                        name=f"I-waitsplit-{n_split}", engine=inst.engine
                    )
                    nop.sync_info = mybir.SyncInfo(on_wait=grp, on_update=[])
                    out.append(nop)
                    n_split += 1
                inst.sync_info = mybir.SyncInfo(
                    on_wait=keep, on_update=list(si.on_update)
                )
                changed = True
            out.append(inst)
        if changed:
            insts[:] = out
    return n_split


def build_nc(tiles) -> bass.Bass:
    """One core's kernel. `tiles` is the list of per-tile batch counts C
    (batch elements per partition); total batch = P * sum(tiles)."""
    SC = sum(tiles)
    n_t = len(tiles)
    nc = bass.Bass()
    # partition-major layout: row p holds batch elements p*SC..(p+1)*SC-1,
    # pred and gt interleaved per element ([pred52 | gt52]) so each tile
    # is ONE contiguous DMA with one completion semaphore; small
    # row-sized runs would halve DMA bandwidth
    jt_ext = nc.declare_dram_parameter("jt", [P, SC * 2 * FU], bf16, isOutput=False)
    out_ext = nc.declare_dram_parameter("out", [1, 1], f32, isOutput=True)
    NFMAX = NPAIR * max(tiles)

    with tile.TileContext(nc) as tc, ExitStack() as ctx:
        ins_pool = ctx.enter_context(tc.tile_pool(name="ins", bufs=6))
        mid_pool = ctx.enter_context(tc.tile_pool(name="mid", bufs=4))
        small_pool = ctx.enter_context(tc.tile_pool(name="small", bufs=4))
        const_pool = ctx.enter_context(tc.tile_pool(name="const", bufs=1))
        psum_pool = ctx.enter_context(tc.tile_pool(name="psum", bufs=1, space="PSUM"))

        ones = const_pool.tile([P, 1], bf16)
        nc.vector.memset(ones[:], 1.0)
        # bf16-rounded joints can collide -> exact-zero bones -> den=0;
        # ln(den+eps) keeps those pairs at t = 0*huge = 0 instead of NaN
        eps = const_pool.tile([P, 1], f32)
        nc.vector.memset(eps[:], 1e-30)

        # PSUM accumulators for the batch reduction, <=512 f32 per bank.
        psums = []
        off = 0
        while off < NFMAX:
            w = min(512, NFMAX - off)
            ps = psum_pool.tile([1, w], f32, name=f"ps{off}", tag=f"ps{off}")
            nc.vector.memset(ps[:], 0.0)
            psums.append((off, w, ps))
            off += w
        last_user = {}
        for i, C in enumerate(tiles):
            for k, (poff, w, ps) in enumerate(psums):
                if NPAIR * C > poff:
                    last_user[k] = i

        st = {}
        b0 = 0

        def emit_a(i):
            nonlocal b0
            C = tiles[i]
            FD = C * 2 * FU
            jv = jt_ext[:, b0 : b0 + FD]
            b0 += FD

            # bf16 uv landing tile; rows alternate pred/gt per element
            u = ins_pool.tile([P, FD], bf16, tag="u")
            nc.sync.dma_start(out=u[:], in_=jv)
            uv = u[:].rearrange("p (r j k) -> p r j k", j=J2, k=2)

            # bone gathers on DVE (all packed 4B uv pairs). The root
            # fan-out uses the host-packed ghost copies of joint 0
            # (columns 21:26), so no broadcast operand is needed --
            # a stride-0 operand runs 3x slower on DVE.
            dc = mid_pool.tile([P, 2 * C, NPAIR, 2], bf16, tag="dc")
            subs = [
                (0, uv[:, :, 21:26, :], uv[:, :, 1:6, :]),
                (5, uv[:, :, 1:6, :], uv[:, :, 6:19:3, :]),
                (10, uv[:, :, 6:19:3, :], uv[:, :, 7:20:3, :]),
                (15, uv[:, :, 7:20:3, :], uv[:, :, 8:21:3, :]),
            ]
            for s0, in0, in1 in subs:
                nc.vector.tensor_sub(out=dc[:, :, s0 : s0 + 5, :], in0=in0, in1=in1)
            st[i] = {"C": C, "uv": uv, "dc": dc}

        def emit_prod(i):
            # pred bones are even rows, gt bones odd rows (outer stride
            # does not affect the DVE 2x mode)
            d = st[i]
            C, dc = d["C"], d["dc"]
            pr = mid_pool.tile([P, C, NPAIR, 2], bf16, tag="pr")
            nc.vector.tensor_mul(
                out=pr[:],
                in0=dc[:, 0 : 2 * C : 2],
                in1=dc[:, 1 : 2 * C : 2],
            )
            d["pr"] = pr

        def emit_m(i):
            d = st[i]
            C, dc = d["C"], d["dc"]
            s = mid_pool.tile([P, 2, 2 * C, NPAIR], bf16, tag="s")
            nc.scalar.activation(
                out=s[:], in_=dc[:].rearrange("p c q k -> p k c q"), func=AF.Square
            )
            d["s"] = s

        def emit_b1(i):
            # nadd FIRST in the DVE cycle (its input s is a cycle old)
            # so Pool's den fires early instead of convoying behind the
            # whole DVE queue
            d = st[i]
            C, pr, s = d["C"], d["pr"], d["s"]
            n = small_pool.tile([P, 2 * C, NPAIR], bf16, tag="n")
            nc.vector.tensor_add(
                out=n[:].rearrange("p c q -> p (c q)"),
                in0=s[:, 0].rearrange("p c q -> p (c q)"),
                in1=s[:, 1].rearrange("p c q -> p (c q)"),
            )
            dot = small_pool.tile([P, C, NPAIR], bf16, tag="dot")
            nc.vector.tensor_add(out=dot[:], in0=pr[:, :, :, 0], in1=pr[:, :, :, 1])
            den = small_pool.tile([P, C, NPAIR], bf16, tag="den")
            nc.gpsimd.tensor_mul(
                out=den[:],
                in0=n[:, 0 : 2 * C : 2],
                in1=n[:, 1 : 2 * C : 2],
            )
            d["dot"], d["den"] = dot, den

        def emit_abs(i):
            d = st[i]
            C, dot = d["C"], d["dot"]
            a = small_pool.tile([P, NPAIR * C], bf16, tag="a")
            nc.scalar.activation(
                out=a[:], in_=dot[:].rearrange("p c q -> p (c q)"), func=AF.Abs
            )
            d["a"] = a

        def emit_b2(i):
            d = st.pop(i)
            C, den, a = d["C"], d["den"], d["a"]
            NF = NPAIR * C
            lg = small_pool.tile([P, NF], bf16, tag="lg")
            nc.scalar.activation(
                out=lg[:],
                in_=den[:].rearrange("p c q -> p (c q)"),
                func=AF.Ln,
                bias=eps[:],
            )
            e = small_pool.tile([P, NF], bf16, tag="e")
            nc.scalar.activation(out=e[:], in_=lg[:], func=AF.Exp, scale=-0.5)
            t = small_pool.tile([P, NF], bf16, tag="t")
            nc.vector.tensor_mul(out=t[:], in0=a[:], in1=e[:])

            for k, (poff, w, ps) in enumerate(psums):
                if NF <= poff:
                    continue
                ww = min(w, NF - poff)
                nc.tensor.matmul(
                    out=ps[:, 0:ww],
                    lhsT=ones[:],
                    rhs=t[:, poff : poff + ww],
                    start=False,
                    stop=(last_user[k] == i),
                    skip_group_check=True,
                )

        # 5-deep software pipeline:
        # B1(k-3) | A(k)+S0(k) | B2(k-4) | P(k-1) | M(k-2) | ABS(k-3)
        # DVE queue: nadd, dot, subs123, t, prod; Pool: den, sub0;
        # ACT: ln, exp, sq, abs -- every queue head's input is >= one
        # cycle old, so no engine convoys behind another.
        for k in range(n_t + 4):
            if 3 <= k <= n_t + 2:
                emit_b1(k - 3)
            if k < n_t:
                emit_a(k)
            if 4 <= k:
                emit_b2(k - 4)
            if 1 <= k <= n_t:
                emit_prod(k - 1)
            if 2 <= k <= n_t + 1:
                emit_m(k - 2)
            if 3 <= k <= n_t + 2:
                emit_abs(k - 3)

        # Tail: reduce each PSUM bank directly (DVE reads PSUM), then the
        # tiny per-bank sums, then DMA the scalar out
        t3 = const_pool.tile([1, len(psums)], f32)
        for k, (poff, w, ps) in enumerate(psums):
            nc.vector.tensor_reduce(
                out=t3[:, k : k + 1],
                in_=ps[:],
                op=mybir.AluOpType.add,
                axis=mybir.AxisListType.X,
            )
        total = const_pool.tile([1, 1], f32)
        nc.vector.tensor_reduce(
            out=total[:], in_=t3[:], op=mybir.AluOpType.add, axis=mybir.AxisListType.X
        )
        nc.sync.dma_start(out=out_ext[:], in_=total[:])

    return nc


_NC_CACHE: dict = {}

DEFAULT_TILES = (16, 32, 48, 48, 48, 48, 48, 48, 48, 48, 32, 32, 16)


def _get_nc(tiles) -> bass.Bass:
    key = tuple(tiles)
    if key not in _NC_CACHE:
        nc = build_nc(list(tiles))
        _split_excess_waits(nc)
        _NC_CACHE[key] = nc
    return _NC_CACHE[key]


def kernel(jt_uvd_pred, jt_uvd_gt, _tiles=DEFAULT_TILES, _trace: bool = False):
    pred = np.asarray(jt_uvd_pred)
    gt = np.asarray(jt_uvd_gt)
    Btot = pred.shape[0]
    assert pred.shape == (Btot, J, DCOORD) and gt.shape == (Btot, J, DCOORD)
    bl = P * sum(_tiles)
    assert bl * NCORES == Btot, (Btot, _tiles)

    # Host-side shard prep: uv coords only, rounded to bf16 (the device
    # pipeline is bf16 regardless; this also cuts DMA traffic ~3x), with
    # joint 0 replicated 5x per row so the root fan-out subtract needs
    # no broadcast operand, in partition-major [P, SC*FU] layout for
    # contiguous DMA spans.
    sc = sum(_tiles)

    def pack(arr):
        a = np.ascontiguousarray(arr[:, :, :2]).astype(BF16)
        ghost = np.broadcast_to(a[:, 0:1, :], (Btot, 5, 2))
        return np.concatenate([a, ghost], axis=1).reshape(Btot, FU)

    jt = np.empty((Btot, 2 * FU), dtype=BF16)
    jt[:, :FU] = pack(pred)
    jt[:, FU:] = pack(gt)

    nc = _get_nc(_tiles)
    in_maps = []
    for c in range(NCORES):
        sl = slice(c * bl, (c + 1) * bl)
        in_maps.append({"jt": jt[sl].reshape(P, sc * 2 * FU)})
    res = run_bass_kernel_spmd(
        nc, in_maps, core_ids=list(range(NCORES)), trace=_trace
    )
    total = sum(float(res.results[i]["out"][0, 0]) for i in range(NCORES))
    loss = 1.0 - total / (Btot * NPAIR)
    out = np.float32(loss)
    if _trace:
        return out, res
    return out
